# revision 9
# baseline (speedup 1.0000x reference)
"""Trainium2 Bass kernel for nn_DiagnosableGMM (GMM ELBO-style loss).

Math notes (derived from the reference):
  - q_logvar == -1 everywhere  => inv_var = e (scalar)  => a[k,d] = e*phi[d]
    is RANK-ONE.  The x^2 GEMM therefore collapses into per-row scalars that
    the host can fold, and the only X-dependent k-varying term is the linear
    GEMM  L[n,k] = sum_d x[n,d] * B[d,k]  with  B[d,k] = e*phi[d]*mu[k,d].
  - log_p[n,k]    = L[n,k] + rho[k] + q[n]          (q[n] = sum_d v[d] x[n,d]^2)
  - log_joint     = log_p + log_pi                  (pi uniform)
  - The per-k constant gamma[k] = rho[k] + log_pi is folded INTO the GEMM by
    shifting the inputs:  y = x + s  with  B^T s = gamma  (64x64 solve).
    Then  psum[n,k] = sum_d y[n,d] B[d,k] = L[n,k] + gamma[k] = log_joint - q.
  - Since quad >= 0 bounds the exponent (psum in ~[-86, +13] for this data),
    exp(psum) needs NO per-row max subtraction:
        lse_k(log_joint[n,:]) = q[n] + log(sum_k exp(psum[n,k]))
  - Device work per core (N/8 = 32768 rows): thin GEMM (contraction 64),
    ACT exp (PSUM->SBUF, bf16), DVE segmented reduce, PSUM evacuation, DMA.
    Host does only O(N) / O(K*D) reassembly math.
"""

import numpy as np
import ml_dtypes

from concourse import bacc, bass, mybir, tile
from concourse.bass_utils import run_bass_kernel_spmd

# ---------------------------------------------------------------- constants
N, D, K = 262144, 64, 64
NCORES = 8
NS = N // NCORES          # rows per core = 32768
BLOCK = 1024              # rows per PSUM bank-block (8 tiles x 128 rows)
NB = NS // BLOCK          # 32 blocks per core
G = 4                     # blocks per input DMA (1 MiB chunks)
NG = NB // G              # 8 input DMA groups

LOG2PI = float(np.log(2.0 * np.pi))
PRIOR_LOGVAR0 = -2.0
E1 = float(np.exp(1.0))

F32 = mybir.dt.float32
BF16 = mybir.dt.bfloat16

# number of the 8 sub-tiles of each PSUM bank evacuated by ScalarE (rest DVE)
ACT_EVAC = 5

LAST_RESULT = None        # BassKernelResults of the most recent device run
TRACE = False             # set True (e.g. from test.py) to capture an NTFF trace

_NC_CACHE = None


def _build_bass():
    """Device program (identical for all 8 cores; data differs)."""
    nc = bacc.Bacc(None, target_bir_lowering=False)

    # inputs: pre-shifted, transposed, tile-permuted X;  GEMM coefficients
    yt = nc.declare_dram_parameter("yt", [NG, D, G, 8, 128], F32, isOutput=False)
    bm = nc.declare_dram_parameter("bm", [D, K], F32, isOutput=False)
    # outputs: raw GEMM result (= log_joint - q), and per-row exp-sums
    lp = nc.declare_dram_parameter("lp", [NB, 128, 8, K], F32, isOutput=True)
    ss = nc.declare_dram_parameter("ss", [128, NB, 8], F32, isOutput=True)

    with tile.TileContext(nc) as tc:
        with (
            tc.tile_pool(name="const", bufs=1) as cpool,
            tc.tile_pool(name="inp", bufs=3) as ipool,
            tc.tile_pool(name="epool", bufs=4) as epool,
            tc.tile_pool(name="opool", bufs=4) as opool,
            tc.tile_pool(name="slab", bufs=1) as spool,
            tc.tile_pool(name="ps", bufs=6, space="PSUM") as pspool,
        ):
            bmt = cpool.tile([D, K], F32)
            nc.sync.dma_start(out=bmt[:], in_=bm[:])

            slab = spool.tile([128, NB, 8], F32)

            for g in range(NG):
                ytile = ipool.tile([D, G, 8, 128], F32)
                nc.sync.dma_start(out=ytile[:], in_=yt[g])
                for bb in range(G):
                    b = g * G + bb
                    psum = pspool.tile([128, 8, K], F32)
                    for j in range(8):
                        # psum[:, j, :] = ytile[:, bb, j, :].T @ bmt
                        nc.tensor.matmul(
                            psum[:, j, :],
                            ytile[:, bb, j, :],
                            bmt[:],
                            start=True,
                            stop=True,
                        )
                    et = epool.tile([128, 8, K], BF16)
                    nc.scalar.activation(
                        et[:], psum[:], mybir.ActivationFunctionType.Exp
                    )
                    nc.vector.reduce_sum(
                        slab[:, b, :], et[:], axis=mybir.AxisListType.X
                    )
                    ot = opool.tile([128, 8, K], F32)
                    nc.scalar.copy(ot[:], psum[:])
                    nc.sync.dma_start(out=lp[b], in_=ot[:])

            nc.sync.dma_start(out=ss[:], in_=slab[:])

    nc.finalize()
    return nc


def _get_nc():
    global _NC_CACHE
    if _NC_CACHE is None:
        _NC_CACHE = _build_bass()
    return _NC_CACHE


def kernel(X, u_noise, phi_logits, q_mu, q_logvar, pi_logits, prior_phi_probs):
    global LAST_RESULT
    X = np.asarray(X)
    u = np.asarray(u_noise, dtype=np.float64)
    pl = np.asarray(phi_logits, dtype=np.float64)
    qmu = np.asarray(q_mu, dtype=np.float64)
    qlv = np.clip(np.asarray(q_logvar, dtype=np.float64), -5.0, 5.0)
    pil = np.asarray(pi_logits, dtype=np.float64)
    pphi = np.asarray(prior_phi_probs, dtype=np.float64)

    # ---------------- host-side O(K*D) parameter math (float64) ----------
    gumbel = -np.log(-np.log(u + 1e-9) + 1e-9)
    phi = 1.0 / (1.0 + np.exp(-(pl + gumbel)))          # (D,)
    inv_var = np.exp(-qlv)                               # (K,D) == e for this model
    a = phi[None, :] * inv_var                           # (K,D), rank-one in practice
    B = (a * qmu).T                                      # (D,K): B[d,k]=a[k,d]*mu[k,d]

    const_k = (phi[None, :] * (LOG2PI + qlv)).sum(1)     # (K,)
    acp = (a * qmu**2).sum(1)                            # (K,)  sum_d a*mu^2
    inv_var0 = float(np.exp(-PRIOR_LOGVAR0))
    bg_const = -0.5 * ((1.0 - phi) * (LOG2PI + PRIOR_LOGVAR0)).sum()

    pi = np.exp(pil - pil.max())
    pi = pi / pi.sum()
    log_pi = np.log(pi + 1e-9)                           # (K,)

    rho = -0.5 * const_k - 0.5 * acp + bg_const          # (K,)
    gamma = rho + log_pi                                 # (K,)

    # the rank-one structure of `a` is what the device kernel relies on
    assert np.abs(a - a[0:1]).max() <= 1e-5 * np.abs(a).max(), (
        "q_logvar is not constant; rank-one decomposition invalid"
    )

    # fold gamma into the GEMM: y = x + s with B^T s = gamma
    B32 = B.astype(np.float32)
    s = np.linalg.solve(B32.T.astype(np.float64), gamma)

    # per-row quadratic scalars (x^2 terms; rank-one 'a' makes them k-free)
    v = -0.5 * (a[0] + inv_var0 * (1.0 - phi))           # (D,)
    Xf = X.astype(np.float64)
    q = (Xf * Xf) @ v                                    # (N,)

    # ---------------- device inputs ------------------------------------
    Y = (Xf + s[None, :]).astype(np.float32)             # (N, D)

    in_maps = []
    for c in range(NCORES):
        ys = Y[c * NS:(c + 1) * NS]                      # (NS, D)
        # row r = 1024*b + 8*p + j  ->  arr[g, d, bb, j, p]
        arr = ys.reshape(NG, G, 128, 8, D).transpose(0, 4, 1, 3, 2)
        in_maps.append({
            "yt": np.ascontiguousarray(arr),
            "bm": B32,
        })

    nc = _get_nc()
    res = run_bass_kernel_spmd(nc, in_maps, list(range(NCORES)), trace=TRACE)
    LAST_RESULT = res

    # ---------------- host-side reassembly ------------------------------
    psum_full = np.empty((N, K), dtype=np.float32)
    S_full = np.empty((N,), dtype=np.float64)
    for c in range(NCORES):
        out = res.results[c]
        psum_full[c * NS:(c + 1) * NS] = out["lp"].reshape(NS, K)
        S_full[c * NS:(c + 1) * NS] = (
            out["ss"].transpose(1, 0, 2).reshape(NS).astype(np.float64)
        )

    # log_p = psum + q - log_pi   (fp32 output)
    log_p = (
        psum_full.astype(np.float64) + q[:, None] - log_pi[None, :]
    ).astype(np.float32)

    # log-likelihood:  lse_n = q_n + log(S_n)
    ll = (q + np.log(S_full)).sum()

    q_phi = np.clip(1.0 / (1.0 + np.exp(-pl)), 1e-6, 1.0 - 1e-6)
    p_phi = np.clip(pphi, 1e-6, 1.0 - 1e-6)
    kl_phi = (
        q_phi * (np.log(q_phi) - np.log(p_phi))
        + (1.0 - q_phi) * (np.log(1.0 - q_phi) - np.log(1.0 - p_phi))
    ).sum() * N

    loss = -ll + kl_phi
    return (
        np.float32(loss),
        q_phi.astype(np.float32),
        log_p,
    )


# revision 15
# speedup vs baseline: 1.9995x; 1.9995x over previous
"""Trainium2 Bass kernel for nn_DiagnosableGMM (GMM ELBO-style loss).

Math notes (derived from the reference):
  - q_logvar == -1 everywhere  => inv_var = e (scalar)  => a[k,d] = e*phi[d]
    is RANK-ONE.  The x^2 GEMM therefore collapses into per-row scalars that
    the host can fold, and the only X-dependent k-varying term is the linear
    GEMM  L[n,k] = sum_d x[n,d] * B[d,k]  with  B[d,k] = e*phi[d]*mu[k,d].
  - log_p[n,k]    = L[n,k] + rho[k] + q[n]          (q[n] = sum_d v[d] x[n,d]^2)
  - log_joint     = log_p + log_pi                  (pi uniform)
  - The per-k constant gamma[k] = rho[k] + log_pi is folded INTO the GEMM by
    shifting the inputs:  y = x + s  with  B^T s = gamma  (64x64 solve).
    Then  psum[n,k] = sum_d y[n,d] B[d,k] = L[n,k] + gamma[k] = log_joint - q.
  - Since quad >= 0 bounds the exponent (psum in ~[-86, +13] for this data),
    exp(psum) needs NO per-row max subtraction:
        lse_k(log_joint[n,:]) = q[n] + log(sum_k exp(psum[n,k]))
  - Device work per core (N/8 = 32768 rows): thin GEMM (contraction 64),
    ACT exp (PSUM->SBUF, bf16), DVE segmented reduce, PSUM evacuation, DMA.
    Host does only O(N) / O(K*D) reassembly math.
"""

import numpy as np
import ml_dtypes

from concourse import bacc, bass, mybir, tile
from concourse.bass_utils import run_bass_kernel_spmd

# ---------------------------------------------------------------- constants
N, D, K = 262144, 64, 64
NCORES = 8
NS = N // NCORES          # rows per core = 32768
BLOCK = 1024              # rows per PSUM bank-block (8 tiles x 128 rows)
NB = NS // BLOCK          # 32 blocks per core
G = 4                     # blocks per input DMA (1 MiB chunks)
NG = NB // G              # 8 input DMA groups

LOG2PI = float(np.log(2.0 * np.pi))
PRIOR_LOGVAR0 = -2.0
E1 = float(np.exp(1.0))

F32 = mybir.dt.float32
F16 = mybir.dt.float16
BF16 = mybir.dt.bfloat16

C = D + 2  # contraction depth: 64 data rows + 2 ones-rows carrying the bias

# number of the 8 sub-tiles of each PSUM bank evacuated by ScalarE (rest DVE)
ACT_EVAC = 3

LAST_RESULT = None        # BassKernelResults of the most recent device run
TRACE = False             # set True (e.g. from test.py) to capture an NTFF trace

_NC_CACHE = None


def _build_bass():
    """Device program (identical for all 8 cores; data differs)."""
    nc = bacc.Bacc(None, target_bir_lowering=False)

    # inputs: transposed, tile-permuted X (+2 ones rows);  GEMM coefficients
    yt = nc.declare_dram_parameter("yt", [NG, C, G, 8, 128], F16, isOutput=False)
    bm = nc.declare_dram_parameter("bm", [C, K], F16, isOutput=False)
    # outputs: raw GEMM result (= log_joint - q), and per-row exp-sums
    lp = nc.declare_dram_parameter("lp", [NB, 128, 8, K], F32, isOutput=True)
    ss = nc.declare_dram_parameter("ss", [128, NB, 8], F32, isOutput=True)

    with tile.TileContext(nc) as tc:
        with (
            tc.tile_pool(name="const", bufs=1) as cpool,
            tc.tile_pool(name="inp", bufs=3) as ipool,
            tc.tile_pool(name="epool", bufs=4) as epool,
            tc.tile_pool(name="opool", bufs=4) as opool,
            tc.tile_pool(name="slab", bufs=1) as spool,
            tc.tile_pool(name="ps", bufs=6, space="PSUM") as pspool,
        ):
            bmt = cpool.tile([C, K], F16)
            nc.sync.dma_start(out=bmt[:], in_=bm[:])

            slab = spool.tile([128, NB, 8], F32)

            for g in range(NG):
                ytile = ipool.tile([C, G, 8, 128], F16)
                nc.sync.dma_start(out=ytile[:], in_=yt[g])
                for bb in range(G):
                    b = g * G + bb
                    psum = pspool.tile([128, 8, K], F32)
                    for j in range(8):
                        # psum[:, j, :] = ytile[:, bb, j, :].T @ bmt
                        nc.tensor.matmul(
                            psum[:, j, :],
                            ytile[:, bb, j, :],
                            bmt[:],
                            start=True,
                            stop=True,
                        )
                    et = epool.tile([128, 8, K], BF16)
                    nc.scalar.activation(
                        et[:], psum[:], mybir.ActivationFunctionType.Exp
                    )
                    nc.vector.reduce_sum(
                        slab[:, b, :], et[:], axis=mybir.AxisListType.X
                    )
                    ot = opool.tile([128, 8, K], F32)
                    nc.scalar.copy(ot[:, :ACT_EVAC, :], psum[:, :ACT_EVAC, :])
                    nc.vector.tensor_copy(ot[:, ACT_EVAC:, :], psum[:, ACT_EVAC:, :])
                    nc.sync.dma_start(out=lp[b], in_=ot[:])

            nc.sync.dma_start(out=ss[:], in_=slab[:])

    nc.finalize()
    return nc


def _get_nc():
    global _NC_CACHE
    if _NC_CACHE is None:
        _NC_CACHE = _build_bass()
    return _NC_CACHE


def kernel(X, u_noise, phi_logits, q_mu, q_logvar, pi_logits, prior_phi_probs):
    global LAST_RESULT
    X = np.asarray(X)
    u = np.asarray(u_noise, dtype=np.float64)
    pl = np.asarray(phi_logits, dtype=np.float64)
    qmu = np.asarray(q_mu, dtype=np.float64)
    qlv = np.clip(np.asarray(q_logvar, dtype=np.float64), -5.0, 5.0)
    pil = np.asarray(pi_logits, dtype=np.float64)
    pphi = np.asarray(prior_phi_probs, dtype=np.float64)

    # ---------------- host-side O(K*D) parameter math (float64) ----------
    gumbel = -np.log(-np.log(u + 1e-9) + 1e-9)
    phi = 1.0 / (1.0 + np.exp(-(pl + gumbel)))          # (D,)
    inv_var = np.exp(-qlv)                               # (K,D) == e for this model
    a = phi[None, :] * inv_var                           # (K,D), rank-one in practice
    B = (a * qmu).T                                      # (D,K): B[d,k]=a[k,d]*mu[k,d]

    const_k = (phi[None, :] * (LOG2PI + qlv)).sum(1)     # (K,)
    acp = (a * qmu**2).sum(1)                            # (K,)  sum_d a*mu^2
    inv_var0 = float(np.exp(-PRIOR_LOGVAR0))
    bg_const = -0.5 * ((1.0 - phi) * (LOG2PI + PRIOR_LOGVAR0)).sum()

    pi = np.exp(pil - pil.max())
    pi = pi / pi.sum()
    log_pi = np.log(pi + 1e-9)                           # (K,)

    rho = -0.5 * const_k - 0.5 * acp + bg_const          # (K,)
    gamma = rho + log_pi                                 # (K,)

    # the rank-one structure of `a` is what the device kernel relies on
    assert np.abs(a - a[0:1]).max() <= 1e-5 * np.abs(a).max(), (
        "q_logvar is not constant; rank-one decomposition invalid"
    )

    # gamma rides in the GEMM as two extra contraction rows (hi/lo fp16 split)
    ghi = gamma.astype(np.float16)
    glo = (gamma - ghi.astype(np.float64)).astype(np.float16)
    bm16 = np.concatenate(
        [B.astype(np.float16), ghi[None, :], glo[None, :]], axis=0
    )                                                    # (C, K)

    # per-row quadratic scalars (x^2 terms; rank-one 'a' makes them k-free)
    v = -0.5 * (a[0] + inv_var0 * (1.0 - phi))           # (D,)
    Xf = X.astype(np.float64)
    q = (Xf * Xf) @ v                                    # (N,)

    # ---------------- device inputs ------------------------------------
    Y = X.astype(np.float16)                             # (N, D)

    in_maps = []
    for c in range(NCORES):
        ys = Y[c * NS:(c + 1) * NS]                      # (NS, D)
        # row r = 1024*b + 8*p + j  ->  arr[g, d, bb, j, p]
        arr = np.ones((NG, C, G, 8, 128), dtype=np.float16)
        arr[:, :D] = ys.reshape(NG, G, 128, 8, D).transpose(0, 4, 1, 3, 2)
        in_maps.append({
            "yt": arr,
            "bm": bm16,
        })

    nc = _get_nc()
    res = run_bass_kernel_spmd(nc, in_maps, list(range(NCORES)), trace=TRACE)
    LAST_RESULT = res

    # ---------------- host-side reassembly ------------------------------
    psum_full = np.empty((N, K), dtype=np.float32)
    S_full = np.empty((N,), dtype=np.float64)
    for c in range(NCORES):
        out = res.results[c]
        psum_full[c * NS:(c + 1) * NS] = out["lp"].reshape(NS, K)
        S_full[c * NS:(c + 1) * NS] = (
            out["ss"].transpose(1, 0, 2).reshape(NS).astype(np.float64)
        )

    # log_p = psum + q - log_pi   (fp32 output)
    log_p = (
        psum_full.astype(np.float64) + q[:, None] - log_pi[None, :]
    ).astype(np.float32)

    # log-likelihood:  lse_n = q_n + log(S_n)
    ll = (q + np.log(S_full)).sum()

    q_phi = np.clip(1.0 / (1.0 + np.exp(-pl)), 1e-6, 1.0 - 1e-6)
    p_phi = np.clip(pphi, 1e-6, 1.0 - 1e-6)
    kl_phi = (
        q_phi * (np.log(q_phi) - np.log(p_phi))
        + (1.0 - q_phi) * (np.log(1.0 - q_phi) - np.log(1.0 - p_phi))
    ).sum() * N

    loss = -ll + kl_phi
    return (
        np.float32(loss),
        q_phi.astype(np.float32),
        log_p,
    )
